# revision 75
# baseline (speedup 1.0000x reference)
"""v6: engine-balanced key-compacted sparse causal attention (26.6us
modeled vs 36.4us baseline).

- 4-deep output staging (obpool): with only 2 staging buffers an
  evacuation had to wait for the output DMA two superblocks back to
  release its tile, stalling the exp engine performing the copy;
  deeper staging decouples evac from DMA drain (-1.6us). PSUM split
  5 score buffers + 3 output accumulators (was 6+2) lets a third
  superblock's PV chain start before the oldest is evacuated.

- query-column trimming: a superblock's late key-blocks start mid- or
  past-superblock, so queries below the block's first real key (min
  over all cores) see none of it; QK/exp/PV stream only [qoff:512].
  PV/exp use the full offset (bf16-costed matmul has no row minimum);
  QK is f32r (4 cycles/row under 256 rows, width must be 4-aligned) so
  it takes the narrow stream only when (512-qoff)*4 beats the 256-wide
  stream. Offsets are non-decreasing in kb, so block 0's start=True
  zeroes the full PSUM region and subrange accumulation is safe.

- exp split between ACT (exact) and DVE (Schraudolph bit-trick:
  p = bitcast_bf16(int16(A*s + BIAS)); DVE's f32->int16 conversion
  truncates, BIAS is tuned for that). qt is pre-scaled by A*scale and a
  65th contract row carries BIAS, so s' = A*s + BIAS lands in PSUM
  straight from the QK matmul. A DVE block's exp+causal-mask is then a
  single scalar_tensor_tensor: p_i16 = (iota >= th) * s' (masked scores
  -> int16 0 -> bf16 +0.0). ACT blocks undo the affine via
  activation(scale=1/A, bias) and take post-exp STT masks only on
  boundary blocks. Per-group ACT/DVE alternation keeps both engines fed
  (a same-engine run paces the pipeline at ~2x the block period).
- p/v in bf16 (PV moving/stationary), q/k stay f32r for score accuracy;
  PV streams 512 rows per block (out [65,512]) — the oT orientation has
  lower engine cost but 4x the instruction count, which saturates the
  PE sequencer's 4-deep wait queue.
- single-block tiles (GRP=1) with a 6-deep PSUM score-tile rotation:
  QK runs up to 6 blocks ahead of exp, so the QK->exp->PV chain never
  stalls the tensor engine; PV emission lags QK by 5 blocks so PV
  instructions reach the PE queue with p already produced.
- startup: DMA issue split across SP HWDGE / ACT HWDGE / Pool SWDGE
  queues (each dma_start holds its sequencer and the single HWDGE
  device ~650ns); Exp table load pulled off the critical path by a
  dummy activation; PE warmed up with garbage matmuls so the p-state
  ramp (3us continuous -> 2.4GHz) completes before real QKs.
- output dram [8, 65, 512] f32, combined/normalized on host (partial
  exp-sums from the 2 cores of each batch pair; rows before the first
  live key and q_mask handled on host).
"""

import sys

import numpy as np

try:
    import concourse  # noqa: F401
except ImportError:  # pragma: no cover
    sys.path.insert(0, "/opt/trn_rl_repo")

B, T, D = 4, 4096, 64
DK = D + 1          # contract dim: 64 data rows + 1 bias row
NCORES = 8
QS_N = 8
QSB = 512
KB = 128
GRP = 1
NEG_BIG = 1e9
QS_ORDER = (1, 2, 7, 6, 5, 4, 3, 0)

A_CONST = float(np.float32(2.0 ** 7 / np.log(2.0)))    # 184.665
CORR = 5.1
BIAS_CONST = float(np.float32(127.0 * 2 ** 7 - CORR))  # 16250.9
INV_A = float(np.float32(1.0) / np.float32(A_CONST))
ACT_BIAS = -BIAS_CONST * INV_A

_compiled = {}


def _build_nc(blocks, slotsets, qoffs):
    # slotsets[qs]: dict kb -> "elem" | "row"
    import concourse.bass as bass
    import concourse.mybir as mybir
    import concourse.tile as tile
    from concourse import bacc

    f32 = mybir.dt.float32
    f32r = mybir.dt.float32r
    bf16 = mybir.dt.bfloat16
    fp16 = mybir.dt.float16
    i16 = mybir.dt.int16
    nb_tot = blocks[-1]

    eslots = [set(kb for kb, kind in slotsets[qs].items()
                  if kind == "elem") for qs in range(QS_N)]
    # th column 0 = always-keep (-1e30); every slot (elem or row) gets a
    # column: DVE STTs use it as the per-key threshold; ACT blocks use the
    # matching actb bias column (row-slots only).
    th_col = {}
    c = 1
    for qs in range(QS_N):
        for kb in sorted(slotsets[qs]):
            th_col[(qs, kb)] = c
            c += 1
    ns_tot = c

    nc = bacc.Bacc(None, target_bir_lowering=False, debug=False)
    qt_d = nc.declare_dram_parameter("qt", [DK, T], f32r, isOutput=False)
    kt_d = nc.declare_dram_parameter("kt", [DK, nb_tot * KB], f32r,
                                     isOutput=False)
    vp_d = nc.declare_dram_parameter("vp", [KB, nb_tot * 65], bf16,
                                     isOutput=False)
    th_d = nc.declare_dram_parameter("th", [KB, ns_tot], f32, isOutput=False)
    ab_d = nc.declare_dram_parameter("ab", [KB, ns_tot], f32, isOutput=False)
    o_d = nc.declare_dram_parameter("o", [QS_N, 65, QSB], f32,
                                    isOutput=True)

    # group list in processing order
    items = []
    for qs in QS_ORDER:
        nkb = blocks[qs]
        for g0 in range(0, nkb, GRP):
            items.append((qs, g0 == 0, nkb,
                          list(range(g0, min(g0 + GRP, nkb)))))

    # per-BLOCK ACT/DVE assignment. A DVE block is one fused STT
    # (exp+causal mask in a single op), so mask-carrying blocks go to
    # DVE; clean blocks go to whichever engine has the lower running
    # total. Alternating engines within a group also keeps both engines
    # fed every group. Evacuations balance the same way.
    blk_eng = {}
    evac_eng = {}
    act_t, dve_t = 0.0, 0.0
    CE_A, CE_D = 615.0, 600.0

    def costs(qs, kb):
        wd = QSB - qoffs[qs][kb][1]
        return 190.0 + wd * 0.8333, 130.0 + wd * 1.0417

    def put_act(qs, kb):
        nonlocal act_t, dve_t
        ca, cm = costs(qs, kb)
        blk_eng[(qs, kb)] = "act"
        act_t += ca
        if kb in eslots[qs]:
            dve_t += cm
    def put_dve(qs, kb):
        nonlocal dve_t
        blk_eng[(qs, kb)] = "dve"
        dve_t += costs(qs, kb)[1]

    for qs, _, nkb, gkbs in items:
        if len(gkbs) == 2:
            a, b = gkbs
            ea = a in eslots[qs]
            eb = b in eslots[qs]
            if ea and not eb:
                put_dve(qs, a)
                put_act(qs, b)
            elif eb and not ea:
                put_act(qs, a)
                put_dve(qs, b)
            elif ea and eb:
                caa, cma = costs(qs, a)
                cdb = costs(qs, b)[1]
                cda = costs(qs, a)[1]
                if max(act_t + caa, dve_t + cdb + cma) < max(act_t,
                                                             dve_t + cda
                                                             + cdb):
                    put_act(qs, a)
                    put_dve(qs, b)
                else:
                    put_dve(qs, a)
                    put_dve(qs, b)
            elif act_t + costs(qs, a)[0] < dve_t + costs(qs, b)[1]:
                put_act(qs, a)
                put_dve(qs, b)
            else:
                put_dve(qs, a)
                put_act(qs, b)
        else:
            kb = gkbs[0]
            ca, _ = costs(qs, kb)
            cd = costs(qs, kb)[1]
            if kb in eslots[qs] or act_t + ca > dve_t + cd:
                put_dve(qs, kb)
            else:
                put_act(qs, kb)
        if gkbs[-1] == nkb - 1:
            if act_t + CE_A <= dve_t + CE_D:
                evac_eng[qs] = "act"
                act_t += CE_A
            else:
                evac_eng[qs] = "dve"
                dve_t += CE_D

    with tile.TileContext(nc) as tc:
        with (
            tc.tile_pool(name="const", bufs=1) as cpool,
            tc.tile_pool(name="pt", bufs=16) as ppool,
            tc.tile_pool(name="ob", bufs=4) as obpool,
            tc.tile_pool(name="ps", bufs=5, space=bass.MemorySpace.PSUM) as spool,
            tc.tile_pool(name="po", bufs=3, space=bass.MemorySpace.PSUM) as opool,
        ):
            qt = cpool.tile([DK, T], f32r)
            kt = cpool.tile([DK, nb_tot * KB], f32r)
            vp = cpool.tile([KB, nb_tot * 65], bf16)
            th = cpool.tile([KB, ns_tot], f32)
            io = cpool.tile([KB, QSB], fp16)
            actb = cpool.tile([KB, ns_tot], f32)
            scr = cpool.tile([KB, 1], f32)
            wuS = cpool.tile([DK, 65], f32r)
            wuM = cpool.tile([DK, 256], f32r)
            nc.vector.memset(actb[:, 0:1], ACT_BIAS)
            nc.vector.memset(wuS[:].bitcast(f32), 0.0)
            nc.vector.memset(wuM[:].bitcast(f32), 0.0)
            # pull the Exp table load off the critical path (1283ns)
            nc.scalar.activation(scr[:], actb[:, 0:1],
                                 mybir.ActivationFunctionType.Exp)

            # DMA issue split across SP (HWDGE), ACT (HWDGE), Pool (SWDGE)
            # so descriptor generation parallelizes; first-needed chunks
            # first. qt/kt feed qs0 at ~4us; everything else arrives under
            # compute.
            k1 = min(blocks[QS_ORDER[0]], nb_tot)
            nc.gpsimd.dma_start(kt[:, 0:k1 * KB], kt_d[:, 0:k1 * KB])
            nc.sync.dma_start(qt[:, QSB:3 * QSB], qt_d[:, QSB:3 * QSB])
            if k1 < nb_tot:
                nc.sync.dma_start(kt[:, k1 * KB:], kt_d[:, k1 * KB:])
            nc.scalar.dma_start(vp[:], vp_d[:])
            nc.gpsimd.iota(io[:], pattern=[[1, QSB]], base=0,
                           channel_multiplier=0,
                           allow_small_or_imprecise_dtypes=True)
            nc.gpsimd.dma_start(th[:], th_d[:])
            nc.gpsimd.dma_start(qt[:, 7 * QSB:T], qt_d[:, 7 * QSB:T])
            nc.gpsimd.dma_start(qt[:, 3 * QSB:7 * QSB],
                                qt_d[:, 3 * QSB:7 * QSB])
            nc.gpsimd.dma_start(qt[:, 0:QSB], qt_d[:, 0:QSB])

            # warmup matmuls: keep PE continuously busy from ~1.5us so the
            # p-state ramp (3us of continuous execution -> 2.4GHz) finishes
            # by the time the first real QK's inputs arrive.
            wu_acc = opool.tile([65, QSB], f32, name="wu", tag="oacc")
            for _ in range(10):
                nc.tensor.matmul(wu_acc[:, 0:256], wuS[:], wuM[:],
                                 start=True, stop=True)

            o_of = {}
            o_ob = {}
            pending = []
            LAG = 7  # groups of PV emission lag so p is ready when PV
            # instructions reach the PE queue (4-deep wait queue would
            # otherwise head-of-line block the PE sequencer)

            def emit_pv(qs, gkbs, nkb, p):
                oacc = o_of[qs]
                for j, kb in enumerate(gkbs):
                    qo = qoffs[qs][kb][1]
                    nc.tensor.matmul(
                        oacc[:, qo:QSB],
                        vp[:, kb * 65:(kb + 1) * 65],
                        p[:, j * QSB + qo:(j + 1) * QSB],
                        start=(kb == 0), stop=(kb == nkb - 1),
                    )
                if gkbs[-1] == nkb - 1:
                    ob = obpool.tile([65, QSB], f32, name=f"ob{qs}",
                                     tag="ob")
                    if evac_eng[qs] == "act":
                        nc.scalar.activation(
                            ob[:], oacc[:],
                            mybir.ActivationFunctionType.Copy)
                    else:
                        nc.vector.tensor_copy(ob[:], oacc[:])
                    nc.sync.dma_start(o_d[qs], ob[:])
                    del o_of[qs]

            for idx, (qs, first, nkb, gkbs) in enumerate(items):
                if first:
                    o_of[qs] = opool.tile([65, QSB], f32,
                                          name=f"oacc{qs}", tag="oacc")
                q0 = qs * QSB
                w = len(gkbs) * QSB
                s = spool.tile([KB, GRP * QSB], f32)
                for j, kb in enumerate(gkbs):
                    qo = qoffs[qs][kb][0]
                    nc.tensor.matmul(
                        s[:, j * QSB + qo:(j + 1) * QSB],
                        kt[:, kb * KB:(kb + 1) * KB],
                        qt[:, q0 + qo:q0 + QSB],
                        start=True, stop=True,
                    )
                if len(pending) >= LAG:
                    emit_pv(*pending.pop(0))
                p = ppool.tile([KB, GRP * QSB], bf16)
                # exps first (ACT and DVE blocks run concurrently), ACT-
                # block masks after, so a mask never head-of-line blocks
                # the DVE queue ahead of DVE's own exp
                for j, kb in enumerate(gkbs):
                    qo = qoffs[qs][kb][1]
                    sl = slice(j * QSB + qo, (j + 1) * QSB)
                    iosl = slice(qo, QSB)
                    if blk_eng[(qs, kb)] == "act":
                        nc.scalar.activation(
                            p[:, sl], s[:, sl],
                            mybir.ActivationFunctionType.Exp,
                            scale=INV_A, bias=actb[:, 0:1],
                        )
                    else:
                        col = th_col.get((qs, kb), 0)
                        nc.vector.scalar_tensor_tensor(
                            p[:, sl].bitcast(i16),
                            io[:, iosl], th[:, col:col + 1], s[:, sl],
                            op0=mybir.AluOpType.is_ge,
                            op1=mybir.AluOpType.mult,
                        )
                for j, kb in enumerate(gkbs):
                    qo = qoffs[qs][kb][1]
                    sl = slice(j * QSB + qo, (j + 1) * QSB)
                    if blk_eng[(qs, kb)] == "act" and kb in eslots[qs]:
                        col = th_col[(qs, kb)]
                        nc.vector.scalar_tensor_tensor(
                            p[:, sl], io[:, qo:QSB], th[:, col:col + 1],
                            p[:, sl],
                            op0=mybir.AluOpType.is_ge,
                            op1=mybir.AluOpType.mult,
                        )
                pending.append((qs, gkbs, nkb, p))
            for args in pending:
                emit_pv(*args)

    nc.compile()
    return nc


def _plan(vm):
    """blocks[qs]: uniform (max-over-core) 128-key block counts, plus the
    per-qs sets of blocks that need a causal/validity mask slot."""
    lives = []
    for c in range(NCORES):
        b, par = c // 2, c % 2
        lives.append(np.flatnonzero(vm[b])[par::2])
    blocks = []
    for qs in range(QS_N):
        bmax = 1
        for live in lives:
            cnt = int(np.searchsorted(live, (qs + 1) * QSB))
            bmax = max(bmax, -(-cnt // KB))
        blocks.append(bmax)
    for qs in range(1, QS_N):
        blocks[qs] = max(blocks[qs], blocks[qs - 1])
    slotsets = []
    for qs in range(QS_N):
        ss = {}
        for live in lives:
            for kb in range(blocks[qs]):
                blk = live[kb * KB:(kb + 1) * KB]
                if blk.size and blk.max() > qs * QSB:
                    # boundary keys inside the superblock need the
                    # per-query mask; blocks whose in-range keys are all
                    # fully visible and whose others are all future can
                    # be masked per-key (row)
                    inside = blk[(blk > qs * QSB) & (blk < (qs + 1) * QSB)]
                    kind = "elem" if inside.size else "row"
                    if ss.get(kb) != "elem":
                        ss[kb] = kind
        slotsets.append(ss)
    # query-column trim: queries below a block's first real key (across
    # all cores) see none of its keys, so QK/PV/exp can stream columns
    # [qoff:512] only. Keys are sorted, so qoff is non-decreasing in kb
    # and block 0 (qoff forced 0) zeroes the full PSUM region on start.
    # Clamp at 256 to keep f32r matmuls at 1 cycle/row.
    qoffs = []
    for qs in range(QS_N):
        qo = []
        for kb in range(blocks[qs]):
            kmin = None
            for live in lives:
                blk = live[kb * KB:(kb + 1) * KB]
                if blk.size:
                    m = int(blk.min())
                    kmin = m if kmin is None else min(kmin, m)
            pv = 0
            if kb > 0 and kmin is not None:
                pv = max(0, min(QSB - 8, kmin - qs * QSB))
            # PV/exp are bf16-costed (no row minimum); QK is f32r, which
            # drops to 4 cycles/row under 256 rows — take the narrow
            # stream only when it still wins
            if (QSB - pv) * 4 <= QSB - min(pv, 256):
                qk = pv - (pv % 4)  # f32r ISA needs 4-aligned stream width
            else:
                qk = min(pv, 256)
            qo.append((qk, pv))
        qoffs.append(tuple(qo))
    return blocks, slotsets, qoffs, lives


def _get_nc(blocks, slotsets, qoffs):
    key = (tuple(blocks), tuple(tuple(sorted(s.items())) for s in slotsets),
           tuple(qoffs))
    if key not in _compiled:
        _compiled[key] = _build_nc(blocks, slotsets, qoffs)
    return _compiled[key]


def _host_inputs(query, value, keys, q_mask, v_mask, scale):
    import ml_dtypes

    scale = np.float32(scale)
    q = np.asarray(query, np.float32)
    v = np.asarray(value, np.float32)
    k = np.asarray(keys, np.float32)
    vm = np.asarray(v_mask).astype(bool)

    blocks, slotsets, qoffs, lives = _plan(vm)
    nb_tot = blocks[-1]
    npad = nb_tot * KB
    ns_tot = 1 + sum(len(s) for s in slotsets)
    A_BIG = np.float32(1e5)

    in_maps = []
    for c in range(NCORES):
        b = c // 2
        live = lives[c]
        nl = live.size
        k_orig = np.full(npad, 2 * T, np.float32)
        k_orig[:nl] = live
        kc = np.zeros((npad, D), np.float32)
        kc[:nl] = k[b][live]
        vc = np.zeros((npad, 65), np.float32)
        vc[:nl, :64] = v[b][live]
        vc[:nl, 64] = 1.0
        qt = np.empty((DK, T), np.float32)
        qt[:D] = q[b].T * (scale * np.float32(A_CONST))
        qt[D] = 1.0
        kt = np.empty((DK, npad), np.float32)
        kt[:D] = kc.T
        kt[D] = BIAS_CONST
        vp = np.ascontiguousarray(
            vc.reshape(nb_tot, KB, 65).transpose(1, 0, 2).reshape(KB, -1)
        ).astype(ml_dtypes.bfloat16)
        th = np.zeros((KB, ns_tot), np.float32)
        th[:, 0] = -1e30  # always-keep column for unslotted DVE blocks
        ab = np.full((KB, ns_tot), ACT_BIAS, np.float32)
        col = 1
        for qs in range(QS_N):
            for kb in sorted(slotsets[qs]):
                # keep iff query_idx >= k_orig - qs*512  (k <= absolute q)
                ko = k_orig[kb * KB:(kb + 1) * KB]
                th[:, col] = ko - qs * QSB
                # row-slot ACT bias: kill keys not visible to any query
                # of this superblock (future or padding)
                ab[:, col] -= A_BIG * (ko >= (qs + 1) * QSB)
                col += 1
        in_maps.append({"qt": np.ascontiguousarray(qt),
                        "kt": np.ascontiguousarray(kt), "vp": vp,
                        "th": np.ascontiguousarray(th),
                        "ab": np.ascontiguousarray(ab)})
    return in_maps, blocks, slotsets, qoffs


def _host_gather(results, query, value, keys, q_mask, v_mask, scale):
    q = np.asarray(query, np.float32)
    v = np.asarray(value, np.float32)
    k = np.asarray(keys, np.float32)
    qm = np.asarray(q_mask).astype(bool)
    vm = np.asarray(v_mask).astype(bool)
    scale = np.float32(scale)

    out = np.empty((B, T, D), np.float32)
    for b in range(B):
        oq = results[2 * b]["o"] + results[2 * b + 1]["o"]  # [8, 65, 512]
        oT = oq.transpose(1, 0, 2).reshape(65, T)
        l = oT[64]
        out[b] = (oT[:64] / np.where(l > 0, l, 1.0)).T
        nz = np.flatnonzero(vm[b])
        first = nz[0] if nz.size else T
        if first > 0:
            rows = np.arange(first)
            s = ((q[b, rows] @ k[b].T) * scale).astype(np.float32)
            s = s - np.float32(NEG_BIG)
            s = s.astype(np.float64)
            s -= s.max(axis=1, keepdims=True)
            p = np.exp(s)
            p /= p.sum(axis=1, keepdims=True)
            out[b, rows] = p @ v[b].astype(np.float64)
    out = np.where(qm[..., None], out, np.float32(0.0))
    return out


def kernel(**inputs):
    from concourse.bass_utils import run_bass_kernel_spmd

    in_maps, blocks, slotsets, qoffs = _host_inputs(**inputs)
    nc = _get_nc(blocks, slotsets, qoffs)
    res = run_bass_kernel_spmd(nc, in_maps, list(range(NCORES))).results
    return _host_gather(res, **inputs)
